# revision 6
# baseline (speedup 1.0000x reference)
"""Mixtral-style MoE (B=4, S=2048, H=2048, I=5632, E=8, top-2, integer softmax)
on 8 Trainium2 NeuronCores.

Strategy: expert-parallel. Routing (integer softmax + top-2 select) is computed
exactly; per-expert token sets are gathered to a fixed capacity C and each core
runs one expert's SwiGLU FFN over its gathered tokens with fp32r (TF32-like)
matmuls on the PE array. Host scatter-adds the weighted per-expert outputs.

Self-contained: hardcodes all shapes; only needs the machine-level concourse /
jax environment.
"""
import os
import sys

if "/opt/trn_rl_repo" not in sys.path:
    sys.path.insert(0, "/opt/trn_rl_repo")

import numpy as np

import concourse.bacc as bacc
import concourse.mybir as mybir
from concourse import tile
from concourse import bass_utils

# problem shapes
B, S, H, I, E = 4, 2048, 2048, 5632, 8
T = B * S                      # 8192 tokens
TOP_K = 2
Q_IN, LUT_MIN, Q_OUT = 128, -1024, 1 << 16

P = 128                        # partitions
TBLK = 512                     # token block (matmul free dim / PSUM bank)
KT = H // P                    # 16 contraction tiles for H
IT = I // P                    # 44 i-tiles
HT = H // P                    # 16 output tiles

_EXP_LUT_CACHE = None


def _exp_lut():
    """Q16 exp LUT, computed with jax exactly as the reference does (jnp.exp
    differs from np.exp in the last ulp for ~half the entries, which shifts
    the int32 truncation)."""
    global _EXP_LUT_CACHE
    if _EXP_LUT_CACHE is None:
        import jax.numpy as jnp
        _EXP_LUT_CACHE = np.asarray(
            (jnp.exp(jnp.arange(LUT_MIN, 1, dtype=jnp.float32) / Q_IN) * Q_OUT
             ).astype(jnp.int32)
        )
    return _EXP_LUT_CACHE


def _route(x2d, w_gate):
    """Exact replication of the reference integer-softmax top-2 routing.

    Returns sel [T, E] bool and wts [T, E] fp32 (renormalized top-2 weights,
    zero for unselected experts)."""
    lg = (x2d.astype(np.float64) @ w_gate.T.astype(np.float64)).astype(np.float32)
    li = np.rint(lg * np.float32(128.0)).astype(np.int32)
    shifted = np.clip(li - li.max(axis=-1, keepdims=True), LUT_MIN, None)
    ev = _exp_lut()[shifted - LUT_MIN]                       # [T, E] int32
    # rank rule == jax.lax.top_k (ties by lower index)
    gt = ev[:, None, :] > ev[:, :, None]                     # [T, e, j]
    eq = ev[:, None, :] == ev[:, :, None]
    jlt = np.arange(E)[None, None, :] < np.arange(E)[None, :, None]
    cnt = (gt | (eq & jlt)).sum(-1)
    sel = cnt < TOP_K
    evf = ev.astype(np.float32)
    den = (evf * sel).sum(-1, keepdims=True)
    wts = np.where(sel, evf / den, np.float32(0.0)).astype(np.float32)
    return sel, wts


_BUILD_CACHE = {}


def _build_ffn(C):
    """Bass program: one expert's SwiGLU FFN over C gathered tokens.

    yt[h, t] = wv[t] * ( (silu(x @ w1.T) * (x @ w3.T)) @ w2.T )[t, h]

    All matmuls fp32r. Layouts (host-prepared):
      xt  [H, C]            x gathered+transposed
      w1p [IT, 128, H]      w1p[it, p, kt*128+i] = w1[it*128+i, kt*128+p]
      w3p [IT, 128, H]      same for w3
      w2p [HT, 128, I]      w2p[ht, p, it*128+i] = w2[ht*128+h?, ...] see host prep
      wv  [128, C]          combine weights replicated across partitions
      yt  [H, C]            output (transposed)
    """
    if C in _BUILD_CACHE:
        return _BUILD_CACHE[C]

    NTB = C // TBLK
    f32, f32r = mybir.dt.float32, mybir.dt.float32r

    IQ = I // 4                # w2 streamed in quarter tiles
    ITQ = IT // 4              # 11 i-tiles per quarter

    nc = bacc.Bacc("TRN2", target_bir_lowering=False, debug=False, num_devices=8)
    xt_d = nc.dram_tensor("xt", [H, C], f32r, kind="ExternalInput").ap()
    w13_d = nc.dram_tensor("w13p", [IT, P, 2 * H], f32r, kind="ExternalInput").ap()
    w2_d = nc.dram_tensor("w2p", [HT, P, I], f32r, kind="ExternalInput").ap()
    wv_d = nc.dram_tensor("wv", [P, C], f32, kind="ExternalInput").ap()
    yt_d = nc.dram_tensor("yt", [H, C], f32, kind="ExternalOutput").ap()

    with tile.TileContext(nc) as tc:
        with (
            tc.tile_pool(name="wv", bufs=2) as wv_pool,
            tc.tile_pool(name="xt", bufs=1) as xt_pool,
            tc.tile_pool(name="w13", bufs=2) as w13_pool,
            tc.tile_pool(name="w2", bufs=4) as w2_pool,
            tc.tile_pool(name="h", bufs=1) as h_pool,
            tc.tile_pool(name="silu", bufs=2) as silu_pool,
            tc.tile_pool(name="ysb", bufs=2) as ysb_pool,
            tc.tile_pool(name="gu_ps", bufs=4, space="PSUM") as gu_pool,
            tc.tile_pool(name="y_ps", bufs=2, space="PSUM") as y_pool,
        ):
            for tb in range(NTB):
                ts = slice(tb * TBLK, (tb + 1) * TBLK)
                wv_t = wv_pool.tile([P, TBLK], f32, tag="wv")
                nc.sync.dma_start(wv_t[:], wv_d[:, ts])
                # activations for this token block: [128, KT, TBLK]
                xt_t = xt_pool.tile([P, KT * TBLK], f32r, tag="xt")
                nc.sync.dma_start(
                    xt_t[:].rearrange("p (kt t) -> p kt t", kt=KT),
                    xt_d[:, ts].rearrange("(kt p) t -> p kt t", p=P),
                )
                xt_v = xt_t[:].rearrange("p (kt t) -> p kt t", kt=KT)

                h_t = h_pool.tile([P, IT * TBLK], f32r, tag="h")
                h_v = h_t[:].rearrange("p (it t) -> p it t", it=IT)

                # ---- phase A: h[i, t] = silu(g) * u over all I tiles ----
                for it in range(IT):
                    w1h = w13_pool.tile([P, H], f32r, tag="w13")
                    w3h = w13_pool.tile([P, H], f32r, tag="w13")
                    nc.sync.dma_start(w1h[:], w13_d[it, :, 0:H])
                    nc.sync.dma_start(w3h[:], w13_d[it, :, H:2 * H])

                    g_ps = gu_pool.tile([P, TBLK], f32, tag="gu")
                    u_ps = gu_pool.tile([P, TBLK], f32, tag="gu")
                    for kt in range(KT):
                        nc.tensor.matmul(
                            g_ps[:], w1h[:, kt * P:(kt + 1) * P],
                            xt_v[:, kt, :],
                            start=(kt == 0), stop=(kt == KT - 1),
                        )
                    for kt in range(KT):
                        nc.tensor.matmul(
                            u_ps[:], w3h[:, kt * P:(kt + 1) * P],
                            xt_v[:, kt, :],
                            start=(kt == 0), stop=(kt == KT - 1),
                        )
                    sg = silu_pool.tile([P, TBLK], f32, tag="silu")
                    nc.scalar.activation(
                        sg[:], g_ps[:], mybir.ActivationFunctionType.Silu
                    )
                    nc.vector.tensor_tensor(
                        h_v[:, it, :], sg[:], u_ps[:], op=mybir.AluOpType.mult
                    )

                # ---- phase B: yt[h, t] = wv[t] * (w2 @ h) ----
                for ht in range(HT):
                    w2q = []
                    for q in range(4):
                        wq = w2_pool.tile([P, IQ], f32r, tag="w2")
                        nc.sync.dma_start(wq[:], w2_d[ht, :, q * IQ:(q + 1) * IQ])
                        w2q.append(wq)
                    y_ps = y_pool.tile([P, TBLK], f32, tag="y")
                    for it in range(IT):
                        wt = w2q[it // ITQ]
                        is_ = slice((it % ITQ) * P, (it % ITQ + 1) * P)
                        nc.tensor.matmul(
                            y_ps[:], wt[:, is_],
                            h_v[:, it, :],
                            start=(it == 0), stop=(it == IT - 1),
                        )
                    y_sb = ysb_pool.tile([P, TBLK], f32, tag="ysb")
                    nc.vector.tensor_tensor(
                        y_sb[:], y_ps[:], wv_t[:], op=mybir.AluOpType.mult
                    )
                    nc.sync.dma_start(yt_d[ht * P:(ht + 1) * P, ts], y_sb[:])

    nc.compile()
    _BUILD_CACHE[C] = nc
    return nc


def _prep_weights(w1, w2, w3):
    """Pretile per-expert weights into SBUF-friendly layouts (all fp32):
      w1p[e][it, p, kt*128+i] = w1[e][it*128+i, kt*128+p]   ([IT, 128, H])
      w3p same
      w2p[e][ht, p, it*128+i] = w2[e][ht*128+h, it*128+p] -> lhsT [i-part, h]
        i.e. w2p[ht, p, it*128+hh] = w2[e][ht*128+hh, it*128+p]
    """
    w13p = np.empty((E, IT, P, 2 * H), np.float32)
    w13p[:, :, :, :H] = w1.reshape(E, IT, P, KT, P).transpose(0, 1, 4, 3, 2).reshape(
        E, IT, P, H)
    w13p[:, :, :, H:] = w3.reshape(E, IT, P, KT, P).transpose(0, 1, 4, 3, 2).reshape(
        E, IT, P, H)
    w2p = np.ascontiguousarray(
        w2.reshape(E, HT, P, IT, P).transpose(0, 1, 4, 3, 2)
    ).reshape(E, HT, P, I)
    return w13p, w2p


def kernel(x, w_gate, w1, w2, w3):
    x = np.asarray(x, dtype=np.float32)
    w_gate = np.asarray(w_gate, dtype=np.float32)
    w1 = np.asarray(w1, dtype=np.float32)
    w2 = np.asarray(w2, dtype=np.float32)
    w3 = np.asarray(w3, dtype=np.float32)

    x2d = x.reshape(T, H)
    sel, wts = _route(x2d, w_gate)
    counts = sel.sum(0)
    C = max(2560, (int(counts.max()) + 63 + TBLK - 1) // TBLK * TBLK)

    w13p, w2p = _prep_weights(w1, w2, w3)

    idxs, in_maps = [], []
    for e in range(E):
        idx = np.nonzero(sel[:, e])[0]
        idxs.append(idx)
        xsel = np.zeros((C, H), np.float32)
        xsel[:len(idx)] = x2d[idx]
        wv = np.zeros(C, np.float32)
        wv[:len(idx)] = wts[idx, e]
        in_maps.append({
            "xt": np.ascontiguousarray(xsel.T),
            "w13p": w13p[e],
            "w2p": w2p[e],
            "wv": np.broadcast_to(wv, (P, C)).copy(),
        })

    nc = _build_ffn(C)
    trace = bool(int(os.environ.get("BASS_MOE_TRACE", "0")))
    res = bass_utils.run_bass_kernel_spmd(
        nc, in_maps, core_ids=list(range(8)), trace=trace
    )
    if trace:
        kernel.last_exec_time_ns = res.exec_time_ns

    out2d = np.zeros((T, H), np.float32)
    for e in range(E):
        idx = idxs[e]
        out2d[idx] += res.results[e]["yt"].T[:len(idx)]
    return out2d.reshape(B, S, H)


kernel.last_exec_time_ns = None


# revision 11
# speedup vs baseline: 1.2565x; 1.2565x over previous
"""Mixtral-style MoE (B=4, S=2048, H=2048, I=5632, E=8, top-2, integer softmax)
on 8 Trainium2 NeuronCores.

Strategy: expert-parallel. Routing (integer softmax + top-2 select) is computed
exactly; per-expert token sets are gathered to a fixed capacity C and each core
runs one expert's SwiGLU FFN over its gathered tokens with fp32r (TF32-like)
matmuls on the PE array. Host scatter-adds the weighted per-expert outputs.

Self-contained: hardcodes all shapes; only needs the machine-level concourse /
jax environment.
"""
import os
import sys

if "/opt/trn_rl_repo" not in sys.path:
    sys.path.insert(0, "/opt/trn_rl_repo")

import numpy as np

import concourse.bacc as bacc
import concourse.mybir as mybir
from concourse import tile
from concourse import bass_utils

# problem shapes
B, S, H, I, E = 4, 2048, 2048, 5632, 8
T = B * S                      # 8192 tokens
TOP_K = 2
Q_IN, LUT_MIN, Q_OUT = 128, -1024, 1 << 16

P = 128                        # partitions
TBLK = 512                     # token block (matmul free dim / PSUM bank)
KT = H // P                    # 16 contraction tiles for H
IT = I // P                    # 44 i-tiles
HT = H // P                    # 16 output tiles

_EXP_LUT_CACHE = None


def _exp_lut():
    """Q16 exp LUT, computed with jax exactly as the reference does (jnp.exp
    differs from np.exp in the last ulp for ~half the entries, which shifts
    the int32 truncation)."""
    global _EXP_LUT_CACHE
    if _EXP_LUT_CACHE is None:
        import jax.numpy as jnp
        _EXP_LUT_CACHE = np.asarray(
            (jnp.exp(jnp.arange(LUT_MIN, 1, dtype=jnp.float32) / Q_IN) * Q_OUT
             ).astype(jnp.int32)
        )
    return _EXP_LUT_CACHE


def _route(x2d, w_gate):
    """Exact replication of the reference integer-softmax top-2 routing.

    Returns sel [T, E] bool and wts [T, E] fp32 (renormalized top-2 weights,
    zero for unselected experts)."""
    lg = (x2d.astype(np.float64) @ w_gate.T.astype(np.float64)).astype(np.float32)
    li = np.rint(lg * np.float32(128.0)).astype(np.int32)
    shifted = np.clip(li - li.max(axis=-1, keepdims=True), LUT_MIN, None)
    ev = _exp_lut()[shifted - LUT_MIN]                       # [T, E] int32
    # rank rule == jax.lax.top_k (ties by lower index)
    gt = ev[:, None, :] > ev[:, :, None]                     # [T, e, j]
    eq = ev[:, None, :] == ev[:, :, None]
    jlt = np.arange(E)[None, None, :] < np.arange(E)[None, :, None]
    cnt = (gt | (eq & jlt)).sum(-1)
    sel = cnt < TOP_K
    evf = ev.astype(np.float32)
    den = (evf * sel).sum(-1, keepdims=True)
    wts = np.where(sel, evf / den, np.float32(0.0)).astype(np.float32)
    return sel, wts


_BUILD_CACHE = {}


def _build_ffn(C):
    """Bass program: one expert's SwiGLU FFN over C gathered tokens.

    yt[h, t] = wv[t] * ( (silu(x @ w1.T) * (x @ w3.T)) @ w2.T )[t, h]

    All matmuls fp32r. Layouts (host-prepared):
      xt  [H, C]            x gathered+transposed
      w1p [IT, 128, H]      w1p[it, p, kt*128+i] = w1[it*128+i, kt*128+p]
      w3p [IT, 128, H]      same for w3
      w2p [HT, 128, I]      w2p[ht, p, it*128+i] = w2[ht*128+h?, ...] see host prep
      wv  [128, C]          combine weights replicated across partitions
      yt  [H, C]            output (transposed)
    """
    if C in _BUILD_CACHE:
        return _BUILD_CACHE[C]

    NTB = C // TBLK
    f32, f32r = mybir.dt.float32, mybir.dt.float32r

    IQ = I // 4                # w2 streamed in quarter tiles
    ITQ = IT // 4              # 11 i-tiles per quarter

    nc = bacc.Bacc("TRN2", target_bir_lowering=False, debug=False, num_devices=8)
    xt_d = nc.dram_tensor("xt", [H, C], f32r, kind="ExternalInput").ap()
    w13_d = nc.dram_tensor("w13p", [IT, P, 2 * H], f32r, kind="ExternalInput").ap()
    w2_d = nc.dram_tensor("w2p", [HT, P, I], f32r, kind="ExternalInput").ap()
    wv_d = nc.dram_tensor("wv", [P, C], f32, kind="ExternalInput").ap()
    yt_d = nc.dram_tensor("yt", [H, C], f32, kind="ExternalOutput").ap()

    with tile.TileContext(nc) as tc:
        with (
            tc.tile_pool(name="wv", bufs=2) as wv_pool,
            tc.tile_pool(name="xt", bufs=1) as xt_pool,
            tc.tile_pool(name="w13", bufs=8) as w13_pool,
            tc.tile_pool(name="w2", bufs=14) as w2_pool,
            tc.tile_pool(name="h", bufs=1) as h_pool,
            tc.tile_pool(name="silu", bufs=2) as silu_pool,
            tc.tile_pool(name="ysb", bufs=2) as ysb_pool,
            tc.tile_pool(name="gu_ps", bufs=4, space="PSUM") as gu_pool,
            tc.tile_pool(name="y_ps", bufs=2, space="PSUM") as y_pool,
        ):
            for tb in range(NTB):
                ts = slice(tb * TBLK, (tb + 1) * TBLK)
                wv_t = wv_pool.tile([P, TBLK], f32, tag="wv")
                nc.sync.dma_start(wv_t[:], wv_d[:, ts])
                # activations for this token block: [128, KT, TBLK]
                xt_t = xt_pool.tile([P, KT * TBLK], f32r, tag="xt")
                nc.sync.dma_start(
                    xt_t[:].rearrange("p (kt t) -> p kt t", kt=KT),
                    xt_d[:, ts].rearrange("(kt p) t -> p kt t", p=P),
                )
                xt_v = xt_t[:].rearrange("p (kt t) -> p kt t", kt=KT)

                h_t = h_pool.tile([P, IT * TBLK], f32r, tag="h")
                h_v = h_t[:].rearrange("p (it t) -> p it t", it=IT)

                # ---- phase A: h[i, t] = silu(g) * u over all I tiles ----
                for it in range(IT):
                    # w1/w3 columns for this i-tile, as 4 quarter tiles so the
                    # pool (bufs=8) always has a full iteration of prefetch
                    wq = []
                    for q in range(4):
                        t_ = w13_pool.tile([P, H // 2], f32r, tag="w13")
                        nc.sync.dma_start(
                            t_[:], w13_d[it, :, q * (H // 2):(q + 1) * (H // 2)]
                        )
                        wq.append(t_)

                    g_ps = gu_pool.tile([P, TBLK], f32, tag="gu")
                    u_ps = gu_pool.tile([P, TBLK], f32, tag="gu")
                    KH = KT // 2
                    for kt in range(KT):
                        wt = wq[kt // KH]
                        nc.tensor.matmul(
                            g_ps[:], wt[:, (kt % KH) * P:(kt % KH + 1) * P],
                            xt_v[:, kt, :],
                            start=(kt == 0), stop=(kt == KT - 1),
                        )
                    for kt in range(KT):
                        wt = wq[2 + kt // KH]
                        nc.tensor.matmul(
                            u_ps[:], wt[:, (kt % KH) * P:(kt % KH + 1) * P],
                            xt_v[:, kt, :],
                            start=(kt == 0), stop=(kt == KT - 1),
                        )
                    sg = silu_pool.tile([P, TBLK], f32, tag="silu")
                    nc.scalar.activation(
                        sg[:], g_ps[:], mybir.ActivationFunctionType.Silu
                    )
                    nc.vector.tensor_tensor(
                        h_v[:, it, :], sg[:], u_ps[:], op=mybir.AluOpType.mult
                    )

                # ---- phase B: yt[h, t] = wv[t] * (w2 @ h) ----
                for ht in range(HT):
                    # w2 columns for this output tile, as 11 tiles of 512
                    # (4 i-tiles each; bufs=14 -> rolling prefetch into next ht)
                    W2T = 512
                    w2q = []
                    for q in range(I // W2T):
                        wq = w2_pool.tile([P, W2T], f32r, tag="w2")
                        nc.sync.dma_start(wq[:], w2_d[ht, :, q * W2T:(q + 1) * W2T])
                        w2q.append(wq)
                    y_ps = y_pool.tile([P, TBLK], f32, tag="y")
                    for it in range(IT):
                        wt = w2q[it // 4]
                        is_ = slice((it % 4) * P, (it % 4 + 1) * P)
                        nc.tensor.matmul(
                            y_ps[:], wt[:, is_],
                            h_v[:, it, :],
                            start=(it == 0), stop=(it == IT - 1),
                        )
                    y_sb = ysb_pool.tile([P, TBLK], f32, tag="ysb")
                    nc.vector.tensor_tensor(
                        y_sb[:], y_ps[:], wv_t[:], op=mybir.AluOpType.mult
                    )
                    nc.sync.dma_start(yt_d[ht * P:(ht + 1) * P, ts], y_sb[:])

    nc.compile()
    _BUILD_CACHE[C] = nc
    return nc


def _prep_weights(w1, w2, w3):
    """Pretile per-expert weights into SBUF-friendly layouts (all fp32):
      w1p[e][it, p, kt*128+i] = w1[e][it*128+i, kt*128+p]   ([IT, 128, H])
      w3p same
      w2p[e][ht, p, it*128+i] = w2[e][ht*128+h, it*128+p] -> lhsT [i-part, h]
        i.e. w2p[ht, p, it*128+hh] = w2[e][ht*128+hh, it*128+p]
    """
    w13p = np.empty((E, IT, P, 2 * H), np.float32)
    w13p[:, :, :, :H] = w1.reshape(E, IT, P, KT, P).transpose(0, 1, 4, 3, 2).reshape(
        E, IT, P, H)
    w13p[:, :, :, H:] = w3.reshape(E, IT, P, KT, P).transpose(0, 1, 4, 3, 2).reshape(
        E, IT, P, H)
    w2p = np.ascontiguousarray(
        w2.reshape(E, HT, P, IT, P).transpose(0, 1, 4, 3, 2)
    ).reshape(E, HT, P, I)
    return w13p, w2p


def kernel(x, w_gate, w1, w2, w3):
    x = np.asarray(x, dtype=np.float32)
    w_gate = np.asarray(w_gate, dtype=np.float32)
    w1 = np.asarray(w1, dtype=np.float32)
    w2 = np.asarray(w2, dtype=np.float32)
    w3 = np.asarray(w3, dtype=np.float32)

    x2d = x.reshape(T, H)
    sel, wts = _route(x2d, w_gate)
    counts = sel.sum(0)
    C = max(2560, (int(counts.max()) + 63 + TBLK - 1) // TBLK * TBLK)

    w13p, w2p = _prep_weights(w1, w2, w3)

    idxs, in_maps = [], []
    for e in range(E):
        idx = np.nonzero(sel[:, e])[0]
        idxs.append(idx)
        xsel = np.zeros((C, H), np.float32)
        xsel[:len(idx)] = x2d[idx]
        wv = np.zeros(C, np.float32)
        wv[:len(idx)] = wts[idx, e]
        in_maps.append({
            "xt": np.ascontiguousarray(xsel.T),
            "w13p": w13p[e],
            "w2p": w2p[e],
            "wv": np.broadcast_to(wv, (P, C)).copy(),
        })

    nc = _build_ffn(C)
    trace = bool(int(os.environ.get("BASS_MOE_TRACE", "0")))
    res = bass_utils.run_bass_kernel_spmd(
        nc, in_maps, core_ids=list(range(8)), trace=trace
    )
    if trace:
        kernel.last_exec_time_ns = res.exec_time_ns

    out2d = np.zeros((T, H), np.float32)
    for e in range(E):
        idx = idxs[e]
        out2d[idx] += res.results[e]["yt"].T[:len(idx)]
    return out2d.reshape(B, S, H)


kernel.last_exec_time_ns = None
